# revision 46
# baseline (speedup 1.0000x reference)
"""kNN (k=16) + grouped 3->64->64->64 MLP + neighbor max-pool on 8 TRN2 cores.

Pipeline (device does all O(N^2) compute, selection, and MLP flops):
  L1 : S[q,j] = 2<xq,xj> - |xj|^2 on PE as fp32r matmuls (1 cyc/col) with a
       hi/lo fp32r operand decomposition (contraction 11) restoring ~fp32
       accuracy; Act evacuates PSUM to SBUF (Pool cannot read PSUM and only
       DVE can compute max on HW); chunk-16 max via a DVE pairwise max tree
       over contiguous slabs (chunk c = points {c + 256*j}); top-24 chunk
       ids via max8/max_index/match_replace rounds on DVE, software-
       pipelined one block behind the reduce.
  host: gather the 20*16=320 candidate coords per query (index routing).
  L2A: exact squared dists in reference fp32 arithmetic on the 320-wide
       compacted domain (Act squares, Pool subtract chain); exact top-17
       (slot 0 = self) via DVE rounds -> local indices; per-block constants
       preloaded in one DMA, ids/loc written out in one batched DMA.
  L2B: relative coords via matmul-folded subtract, packed 2-point 3-layer
       MLP on PE (fp32r, host-prerounded weights/inputs), neighbor max-pool
       as single-PSUM-input DVE TensorReduce, PE transpose lagged one block,
       final pair max.

Sharding: core c handles batch c//2, query half c%2 (2048 queries each).
"""
import sys
import numpy as np

sys.path.insert(0, "/opt/trn_rl_repo")

import jax
import numpy as _np
from jax.sharding import Mesh, PartitionSpec
from jax.experimental.shard_map import shard_map

import concourse.bacc as bacc
import concourse.mybir as mybir
import concourse.tile as tile
from concourse import bass2jax
from concourse.bass2jax import _bass_exec_p, install_neuronx_cc_hook

F32 = mybir.dt.float32
F32R = mybir.dt.float32r
U16 = mybir.dt.uint16
AX = mybir.AxisListType
OP = mybir.AluOpType
AF = mybir.ActivationFunctionType

def _r32r(a):
    """Round fp32 to fp32r (11 explicit mantissa bits, round-half-up @bit 12)."""
    b = np.ascontiguousarray(a, dtype=np.float32).view(np.uint32)
    b = ((b.astype(np.uint64) + 0x800) & 0xFFFFF000).astype(np.uint32)
    return b.view(np.float32)


def _hilo(a):
    h = _r32r(a)
    return h, _r32r((a.astype(np.float32) - h).astype(np.float32))


B, N, C, K = 4, 4096, 64, 16
KK = K + 1              # 17
CH = 16                 # points per chunk
NCH = N // CH           # 256 chunks; member j of chunk c = point c + 256*j
NSEL = 20               # chunks kept per query (17 guarantee + 3 tie slack)
W = NSEL * CH           # 320 candidate superset per query
NQ = 2048               # queries per core
NBLK = NQ // 128        # 16
NEG = -1.0e30
NCORES = 8
B1P = 576               # Pool share of the B-half first max-tree level

_progs = {}


def _rounds(nc, sp, vals, out_ids, tag):
    """3x (max8 -> max_index -> match_replace) producing 24 ids, mutating vals."""
    for r in range(3):
        m8 = sp.tile([128, 8], F32, tag=f"m8{tag}", name=f"m8{tag}_{r}_{id(vals)}")
        nc.vector.max(out=m8[:], in_=vals)
        nc.vector.max_index(out=out_ids[:, r * 8:(r + 1) * 8], in_max=m8[:],
                            in_values=vals)
        if r < 2:
            nc.vector.match_replace(out=vals, in_to_replace=m8[:], in_values=vals,
                                    imm_value=NEG)


def _build_l1(repeat=1):
    nc = bacc.Bacc("TRN2", target_bir_lowering=False, debug=False,
                   num_devices=NCORES)
    xyzT_d = nc.dram_tensor("xyzT", [11, N], F32R, kind="ExternalInput").ap()
    qT_d = nc.dram_tensor("qT", [11, NQ], F32R, kind="ExternalInput").ap()
    ids_d = nc.dram_tensor("ids", [NQ, 24], U16, kind="ExternalOutput").ap()
    with tile.TileContext(nc) as tc:
        with (
            tc.tile_pool(name="tabs", bufs=1) as tabs,
            tc.tile_pool(name="psum", bufs=1, space="PSUM") as pp,
            tc.tile_pool(name="workbig", bufs=3) as wb,
            tc.tile_pool(name="work", bufs=6) as wp,
            tc.tile_pool(name="small", bufs=8) as sp,
        ):
            xyzT_sb = tabs.tile([11, N], F32R)
            qT_sb = tabs.tile([11, NQ], F32R)
            warm = tabs.tile([11, 512], F32)
            ids_all = tabs.tile([128, NBLK * 24], U16)
            nc.vector.memset(warm[:], 0.0)
            nc.sync.dma_start(out=xyzT_sb[:], in_=xyzT_d[:])
            nc.sync.dma_start(out=qT_sb[:], in_=qT_d[:])
            # ramp the PE to full clock while the inputs stream in
            pswarm = pp.tile([128, 2048], F32, tag="psA", name="pswarm")
            for n in range(10):
                nc.tensor.matmul(pswarm[:, 0:512], warm[:, 0:128], warm[:],
                                 start=True, stop=True)
            c16s = {}
            nblk = repeat * NBLK
            for i in range(nblk):
                ib = i % NBLK
                lhsT = qT_sb[:, ib * 128:(ib + 1) * 128]
                psA = pp.tile([128, 2048], F32, tag="psA", name=f"psA_{i}")
                psB = pp.tile([128, 2048], F32, tag="psB", name=f"psB_{i}")
                for k in range(4):
                    nc.tensor.matmul(psA[:, k * 512:(k + 1) * 512], lhsT,
                                     xyzT_sb[:, k * 512:(k + 1) * 512],
                                     start=True, stop=True)
                for k in range(4):
                    nc.tensor.matmul(psB[:, k * 512:(k + 1) * 512], lhsT,
                                     xyzT_sb[:, (4 + k) * 512:(5 + k) * 512],
                                     start=True, stop=True)
                # HW: Pool can't read PSUM and ops take at most one PSUM
                # operand, so Act (otherwise idle) evacuates PSUM to SBUF and
                # the pair-max tree runs in SBUF. Pairings keep chunk=pt%256.
                cpA = wb.tile([128, 2048], F32, tag="cpA", name=f"cpA_{i}")
                nc.scalar.activation(cpA[:], psA[:], AF.Copy)
                cpB = wb.tile([128, 2048], F32, tag="cpB", name=f"cpB_{i}")
                nc.scalar.activation(cpB[:], psB[:], AF.Copy)
                s1 = wb.tile([128, 2048], F32, tag="s1", name=f"s1_{i}")
                nc.vector.tensor_tensor(s1[:], cpA[:], cpB[:], op=OP.max)
                s2 = wp.tile([128, 1024], F32, tag="s2", name=f"s2_{i}")
                nc.vector.tensor_tensor(s2[:], s1[:, 0:1024], s1[:, 1024:2048],
                                        op=OP.max)
                s3 = wp.tile([128, 512], F32, tag="s3", name=f"s3_{i}")
                nc.vector.tensor_tensor(s3[:], s2[:, 0:512], s2[:, 512:1024],
                                        op=OP.max)
                c16 = wp.tile([128, NCH], F32, tag="c16", name=f"c16_{i}")
                nc.vector.tensor_tensor(c16[:], s3[:, 0:256], s3[:, 256:512],
                                        op=OP.max)
                c16s[i] = c16
                # software pipelining: selection rounds lag the reduce by one
                # block so the DVE->Pool->DVE cycle spans two periods
                if i >= 1:
                    j = i - 1
                    _rounds(nc, sp, c16s.pop(j)[:],
                            ids_all[:, (j % NBLK) * 24:(j % NBLK + 1) * 24], "a")
            j = nblk - 1
            _rounds(nc, sp, c16s.pop(j)[:],
                    ids_all[:, (j % NBLK) * 24:(j % NBLK + 1) * 24], "a")
            nc.sync.dma_start(
                out=ids_d[:].rearrange("(i p) s -> p i s", p=128),
                in_=ids_all[:].rearrange("p (i s) -> p i s", s=24))
    nc.compile()
    return nc


def _build_l2a(repeat=1):
    nc = bacc.Bacc("TRN2", target_bir_lowering=False, debug=False,
                   num_devices=NCORES)
    g_d = nc.dram_tensor("g", [NQ, 3 * W], F32, kind="ExternalInput").ap()
    nq_d = nc.dram_tensor("nq", [NQ, 3], F32, kind="ExternalInput").ap()
    loc_d = nc.dram_tensor("loc", [NQ, 24], U16, kind="ExternalOutput").ap()
    with tile.TileContext(nc) as tc:
        with (
            tc.tile_pool(name="tabs", bufs=1) as tabs,
            tc.tile_pool(name="work", bufs=6) as wp,
            tc.tile_pool(name="small", bufs=6) as sp,
        ):
            nq_all = tabs.tile([128, NBLK * 3], F32)
            loc_all = tabs.tile([128, NBLK * 24], U16)
            zz = tabs.tile([128, W], F32)
            nc.vector.memset(zz[:], 0.0)
            gt0 = wp.tile([128, 3 * W], F32, tag="gt", name="gt_0")
            nc.sync.dma_start(out=gt0[:], in_=g_d[0:128, :])
            nc.sync.dma_start(
                out=nq_all[:].rearrange("p (i c) -> p i c", c=3),
                in_=nq_d[:].rearrange("(i p) c -> p i c", p=128))
            for i in range(repeat * NBLK):
                ib = i % NBLK
                sl = slice(ib * 128, (ib + 1) * 128)
                if i == 0:
                    gt = gt0
                else:
                    gt = wp.tile([128, 3 * W], F32, tag="gt", name=f"gt_{i}")
                    nc.sync.dma_start(out=gt[:], in_=g_d[sl, :])
                nsq = wp.tile([128, 3, W], F32, tag="nsq", name=f"nsq_{i}")
                for c in range(3):
                    nc.scalar.activation(nsq[:, c, :], gt[:, c * W:(c + 1) * W],
                                         AF.Square,
                                         bias=nq_all[:, ib * 3 + c:ib * 3 + c + 1],
                                         scale=1.0)
                # nd = ((0-sq0)-sq1)-sq2, bit-identical to reference order
                nd = wp.tile([128, W], F32, tag="nd", name=f"nd_{i}")
                nc.gpsimd.tensor_tensor(nd[:], zz[:], nsq[:, 0, :], op=OP.subtract)
                nc.gpsimd.tensor_tensor(nd[:], nd[:], nsq[:, 1, :], op=OP.subtract)
                nc.gpsimd.tensor_tensor(nd[:], nd[:], nsq[:, 2, :], op=OP.subtract)
                _rounds(nc, sp, nd[:], loc_all[:, ib * 24:(ib + 1) * 24], "b")
            nc.sync.dma_start(
                out=loc_d[:].rearrange("(i p) s -> p i s", p=128),
                in_=loc_all[:].rearrange("p (i s) -> p i s", s=24))
    nc.compile()
    return nc


def _build_l2b(repeat=1):
    nc = bacc.Bacc("TRN2", target_bir_lowering=False, debug=False,
                   num_devices=NCORES)
    g6_d = nc.dram_tensor("g6", [6, NQ * 8], F32R, kind="ExternalInput").ap()
    xq6_d = nc.dram_tensor("xq6", [6, NQ * 8], F32R, kind="ExternalInput").ap()
    w1_d = nc.dram_tensor("w1b", [6, 128], F32R, kind="ExternalInput").ap()
    w1n_d = nc.dram_tensor("w1nb", [6, 128], F32R, kind="ExternalInput").ap()
    w2_d = nc.dram_tensor("w2b", [128, 128], F32R, kind="ExternalInput").ap()
    w3_d = nc.dram_tensor("w3b", [128, 128], F32R, kind="ExternalInput").ap()
    eye_d = nc.dram_tensor("eye", [128, 128], F32, kind="ExternalInput").ap()
    out_d = nc.dram_tensor("out", [NQ, C], F32, kind="ExternalOutput").ap()
    with tile.TileContext(nc) as tc:
        with (
            tc.tile_pool(name="tabs", bufs=1) as tabs,
            tc.tile_pool(name="psum", bufs=1, space="PSUM") as pp,
            tc.tile_pool(name="psumT", bufs=2, space="PSUM") as ppt,
            tc.tile_pool(name="work", bufs=4) as wp,
            tc.tile_pool(name="small", bufs=4) as sp,
        ):
            w1_sb = tabs.tile([6, 128], F32R)
            w1n_sb = tabs.tile([6, 128], F32R)
            w2_sb = tabs.tile([128, 128], F32R)
            w3_sb = tabs.tile([128, 128], F32R)
            eye_sb = tabs.tile([128, 128], F32)
            g6_sb = tabs.tile([6, NQ * 8], F32R)
            xq6_sb = tabs.tile([6, NQ * 8], F32R)
            fin_all = tabs.tile([128, NBLK * 64], F32)
            for sb, dd in ((w1_sb, w1_d), (w1n_sb, w1n_d), (w2_sb, w2_d),
                           (w3_sb, w3_d), (eye_sb, eye_d), (g6_sb, g6_d),
                           (xq6_sb, xq6_d)):
                nc.sync.dma_start(out=sb[:], in_=dd[:])
            warm = tabs.tile([6, 512], F32)
            nc.vector.memset(warm[:], 0.0)
            pswarm = pp.tile([128, 512], F32, tag="ps10", name="pswarm")
            for n in range(8):
                nc.tensor.matmul(pswarm[:], warm[:, 0:128], warm[:],
                                 start=True, stop=True)
            mxs = {}

            def _tail(j):
                jb = j % NBLK
                pst = ppt.tile([128, 128], F32, tag="pst", name=f"pst_{j}")
                nc.tensor.transpose(pst[:], mxs.pop(j)[:], eye_sb[:])
                mxT = sp.tile([128, 128], F32, tag="mxT", name=f"mxT_{j}")
                nc.scalar.activation(mxT[:], pst[:], AF.Copy)
                nc.vector.tensor_tensor(fin_all[:, jb * 64:(jb + 1) * 64],
                                        mxT[:, 0:64], mxT[:, 64:128], op=OP.max)

            for i in range(repeat * NBLK):
                ib = i % NBLK
                mx = sp.tile([128, 128], F32, tag="mx", name=f"mx_{i}")
                cs = [slice(ib * 1024 + t * 512, ib * 1024 + (t + 1) * 512)
                      for t in range(2)]
                # interleave the two independent halves so their chains run
                # concurrently on different engines
                ps1 = [pp.tile([128, 512], F32, tag=f"ps1{t}", name=f"ps1_{i}_{t}")
                       for t in range(2)]
                for t in range(2):
                    nc.tensor.matmul(ps1[t][:], w1_sb[:], g6_sb[:, cs[t]],
                                     start=True, stop=False)
                    nc.tensor.matmul(ps1[t][:], w1n_sb[:], xq6_sb[:, cs[t]],
                                     start=False, stop=True)
                h1 = [wp.tile([128, 512], F32R, tag=f"h1{t}", name=f"h1_{i}_{t}")
                      for t in range(2)]
                nc.scalar.activation(h1[0][:], ps1[0][:], AF.Relu)
                nc.vector.tensor_scalar(h1[1][:], ps1[1][:], 0.0, scalar2=None,
                                        op0=OP.max)
                ps2 = [pp.tile([128, 512], F32, tag=f"ps2{t}", name=f"ps2_{i}_{t}")
                       for t in range(2)]
                for t in range(2):
                    nc.tensor.matmul(ps2[t][:], w2_sb[:], h1[t][:],
                                     start=True, stop=True)
                h2 = [wp.tile([128, 512], F32R, tag=f"h2{t}", name=f"h2_{i}_{t}")
                      for t in range(2)]
                nc.scalar.activation(h2[0][:], ps2[0][:], AF.Relu)
                nc.scalar.activation(h2[1][:], ps2[1][:], AF.Relu)
                ps3 = [pp.tile([128, 512], F32, tag=f"ps3{t}", name=f"ps3_{i}_{t}")
                       for t in range(2)]
                for t in range(2):
                    nc.tensor.matmul(ps3[t][:], w3_sb[:], h2[t][:],
                                     start=True, stop=True)
                # neighbor max-pool: single-PSUM-input TensorReduce on DVE
                for t in range(2):
                    nc.vector.tensor_reduce(
                        mx[:, t * 64:(t + 1) * 64],
                        ps3[t][:].rearrange("m (q p) -> m q p", p=8),
                        axis=AX.X, op=OP.max)
                mxs[i] = mx
                # transpose+final-max lag one block so the PE stream is never
                # head-of-line blocked on the current block's pool tree
                if i >= 1:
                    _tail(i - 1)
            _tail(repeat * NBLK - 1)
            nc.sync.dma_start(
                out=out_d[:].rearrange("(i p) c -> p i c", p=128),
                in_=fin_all[:].rearrange("p (i c) -> p i c", c=64))
    nc.compile()
    return nc


class _Executor:
    """Cached multi-core PJRT executor for one prebuilt Bass program."""

    def __init__(self, nc):
        install_neuronx_cc_hook()
        self.nc = nc
        part_name = nc.partition_id_tensor.name if nc.partition_id_tensor else None
        in_names, out_names, out_avals, zero_outs = [], [], [], []
        for alloc in nc.m.functions[0].allocations:
            if not isinstance(alloc, mybir.MemoryLocationSet):
                continue
            name = alloc.memorylocations[0].name
            if alloc.kind == "ExternalInput":
                if name != part_name:
                    in_names.append(name)
            elif alloc.kind == "ExternalOutput":
                shape = tuple(alloc.tensor_shape)
                dtype = mybir.dt.np(alloc.dtype)
                out_names.append(name)
                out_avals.append(jax.core.ShapedArray(shape, dtype))
                zero_outs.append(_np.zeros(shape, dtype))
        self.in_names, self.out_names = in_names, out_names
        self.out_avals, self.zero_outs = out_avals, zero_outs
        n_params = len(in_names)
        all_names = in_names + out_names
        if part_name is not None:
            all_names = all_names + [part_name]

        def _body(*args):
            operands = list(args)
            if part_name is not None:
                operands.append(bass2jax.partition_id_tensor())
            return tuple(_bass_exec_p.bind(
                *operands,
                out_avals=tuple(out_avals),
                in_names=tuple(all_names),
                out_names=tuple(out_names),
                lowering_input_output_aliases=(),
                sim_require_finite=True,
                sim_require_nnan=True,
                nc=nc,
            ))

        devices = jax.devices()[:NCORES]
        mesh = Mesh(_np.asarray(devices), ("core",))
        n_outs = len(out_names)
        self._fn = jax.jit(
            shard_map(_body, mesh=mesh,
                      in_specs=(PartitionSpec("core"),) * (n_params + n_outs),
                      out_specs=(PartitionSpec("core"),) * n_outs,
                      check_rep=False),
            donate_argnums=tuple(range(n_params, n_params + n_outs)),
            keep_unused=True,
        )

    def prepare(self, in_maps):
        n = NCORES
        return [
            _np.concatenate([_np.asarray(in_maps[c][name]) for c in range(n)], axis=0)
            for name in self.in_names
        ]

    def run_prepared(self, concat_in):
        n = NCORES
        concat_zeros = [_np.zeros((n * z.shape[0], *z.shape[1:]), z.dtype)
                        for z in self.zero_outs]
        return self._fn(*concat_in, *concat_zeros)

    def __call__(self, in_maps):
        n = NCORES
        outs = self.run_prepared(self.prepare(in_maps))
        outs = [_np.asarray(o) for o in outs]
        return [
            {name: outs[i].reshape(n, *self.out_avals[i].shape)[c]
             for i, name in enumerate(self.out_names)}
            for c in range(n)
        ]


def _get_progs():
    if "l1" not in _progs:
        _progs["l1"] = _Executor(_build_l1())
        _progs["l2a"] = _Executor(_build_l2a())
        _progs["l2b"] = _Executor(_build_l2b())
    return _progs["l1"], _progs["l2a"], _progs["l2b"]


def kernel(xyz, w1, w2, w3, k):
    xyz = np.asarray(xyz, dtype=np.float32)
    w1 = np.asarray(w1, dtype=np.float32)
    w2 = np.asarray(w2, dtype=np.float32)
    w3 = np.asarray(w3, dtype=np.float32)
    assert int(k) == K and xyz.shape == (B, N, 3)
    l1, l2a, l2b = _get_progs()
    cores = list(range(NCORES))

    # ---- L1: coarse chunk selection -------------------------------------
    # hi/lo fp32r decomposition restores ~fp32 scoring accuracy on the PE:
    # score = qh.vh + qh.vl + ql.vh - sqh - sql  (v = 2x, sq = |x|^2)
    xyzT_b = []
    for b in range(B):
        X = xyz[b]
        sq = (X[:, 0] ** 2 + X[:, 1] ** 2 + X[:, 2] ** 2).astype(np.float32)
        v = (2.0 * X.T).astype(np.float32)                   # (3, N)
        vh, vl = _hilo(v)
        sqh, sql = _hilo(sq)
        xyzT_b.append(np.concatenate(
            [vh, vl, vh, sqh[None, :], sql[None, :]]).astype(np.float32))
    in1 = []
    for c in cores:
        b, h = c // 2, c % 2
        Q = xyz[b, h * NQ:(h + 1) * NQ]
        qh, ql = _hilo(Q.T.astype(np.float32))               # (3, NQ)
        ones = -np.ones((1, NQ), np.float32)
        qT = np.concatenate([qh, qh, ql, ones, ones]).astype(np.float32)
        in1.append({"xyzT": xyzT_b[b], "qT": qT})
    r1 = l1(in1)

    # ---- host glue: superset gather (chunk c members = c + 256*j) -------
    sup = []   # per-core (NQ, W) global candidate ids
    in2 = []
    for c in cores:
        b, h = c // 2, c % 2
        ids = r1[c]["ids"][:, :NSEL].astype(np.int64)          # (NQ, 20)
        s = (ids[:, :, None] + (np.arange(CH) * NCH)[None, None, :]).reshape(NQ, W)
        sup.append(s)
        g = xyz[b][s]                                          # (NQ, W, 3)
        g3 = np.ascontiguousarray(g.transpose(0, 2, 1)).reshape(NQ, 3 * W)
        nq3 = -np.ascontiguousarray(xyz[b, h * NQ:(h + 1) * NQ])
        in2.append({"g": g3.astype(np.float32), "nq": nq3.astype(np.float32)})
    r2 = l2a(in2)

    # ---- host glue: final-16 gather ------------------------------------
    w1blkT = np.zeros((6, 128), np.float32)
    w1blkT[0:3, 0:64] = w1.T
    w1blkT[3:6, 64:128] = w1.T
    w2blkT = np.zeros((128, 128), np.float32)
    w2blkT[0:64, 0:64] = w2.T
    w2blkT[64:128, 64:128] = w2.T
    w3blkT = np.zeros((128, 128), np.float32)
    w3blkT[0:64, 0:64] = w3.T
    w3blkT[64:128, 64:128] = w3.T
    eye = np.eye(128, dtype=np.float32)
    in3 = []
    for c in cores:
        b, h = c // 2, c % 2
        loc = r2[c]["loc"].astype(np.int64)            # (NQ, 24)
        glob = np.take_along_axis(sup[c], loc[:, 1:KK], axis=1)  # (NQ, 16)
        g16 = xyz[b][glob]                                     # (NQ, 16, 3)
        gA, gB = g16[:, 0::2, :], g16[:, 1::2, :]
        g6 = np.concatenate([gA, gB], axis=2)                  # (NQ, 8, 6)
        g6 = np.ascontiguousarray(g6.transpose(2, 0, 1)).reshape(6, NQ * 8)
        q = xyz[b, h * NQ:(h + 1) * NQ]
        xq6 = np.repeat(np.concatenate([q, q], axis=1)[:, None, :], 8, axis=1)
        xq6 = np.ascontiguousarray(xq6.transpose(2, 0, 1)).reshape(6, NQ * 8)
        in3.append({"g6": _r32r(g6), "xq6": _r32r(xq6),
                    "w1b": _r32r(w1blkT), "w1nb": _r32r(-w1blkT),
                    "w2b": _r32r(w2blkT), "w3b": _r32r(w3blkT), "eye": eye})
    r3 = l2b(in3)

    out = np.zeros((B, C, N), np.float32)
    for c in cores:
        b, h = c // 2, c % 2
        out[b, :, h * NQ:(h + 1) * NQ] = r3[c]["out"].T
    return out


# revision 48
# speedup vs baseline: 1.0001x; 1.0001x over previous
"""kNN (k=16) + grouped 3->64->64->64 MLP + neighbor max-pool on 8 TRN2 cores.

Pipeline (device does all O(N^2) compute, selection, and MLP flops):
  L1 : S[q,j] = 2<xq,xj> - |xj|^2 on PE as fp32r matmuls (1 cyc/col) with a
       hi/lo fp32r operand decomposition (contraction 11) restoring ~fp32
       accuracy; Act evacuates PSUM to SBUF (Pool cannot read PSUM and only
       DVE can compute max on HW); chunk-16 max via a DVE pairwise max tree
       over contiguous slabs (chunk c = points {c + 256*j}); top-24 chunk
       ids via max8/max_index/match_replace rounds on DVE, software-
       pipelined one block behind the reduce.
  host: gather the 20*16=320 candidate coords per query (index routing).
  L2A: exact squared dists in reference fp32 arithmetic on the 320-wide
       compacted domain (Act squares, Pool subtract chain); exact top-17
       (slot 0 = self) via DVE rounds -> local indices; per-block constants
       preloaded in one DMA, ids/loc written out in one batched DMA.
  L2B: relative coords via matmul-folded subtract, packed 2-point 3-layer
       MLP on PE (fp32r, host-prerounded weights/inputs), neighbor max-pool
       as single-PSUM-input DVE TensorReduce, PE transpose lagged one block,
       final pair max.

Sharding: core c handles batch c//2, query half c%2 (2048 queries each).
"""
import sys
import numpy as np

sys.path.insert(0, "/opt/trn_rl_repo")

import jax
import numpy as _np
from jax.sharding import Mesh, PartitionSpec
from jax.experimental.shard_map import shard_map

import concourse.bacc as bacc
import concourse.mybir as mybir
import concourse.tile as tile
from concourse import bass2jax
from concourse.bass2jax import _bass_exec_p, install_neuronx_cc_hook

F32 = mybir.dt.float32
F32R = mybir.dt.float32r
U16 = mybir.dt.uint16
AX = mybir.AxisListType
OP = mybir.AluOpType
AF = mybir.ActivationFunctionType

def _r32r(a):
    """Round fp32 to fp32r (11 explicit mantissa bits, round-half-up @bit 12)."""
    b = np.ascontiguousarray(a, dtype=np.float32).view(np.uint32)
    b = ((b.astype(np.uint64) + 0x800) & 0xFFFFF000).astype(np.uint32)
    return b.view(np.float32)


def _hilo(a):
    h = _r32r(a)
    return h, _r32r((a.astype(np.float32) - h).astype(np.float32))


B, N, C, K = 4, 4096, 64, 16
KK = K + 1              # 17
CH = 16                 # points per chunk
NCH = N // CH           # 256 chunks; member j of chunk c = point c + 256*j
NSEL = 20               # chunks kept per query (17 guarantee + 3 tie slack)
W = NSEL * CH           # 320 candidate superset per query
NQ = 2048               # queries per core
NBLK = NQ // 128        # 16
NEG = -1.0e30
NCORES = 8
B1P = 576               # Pool share of the B-half first max-tree level

_progs = {}


def _rounds(nc, sp, vals, out_ids, tag):
    """3x (max8 -> max_index -> match_replace) producing 24 ids, mutating vals."""
    for r in range(3):
        m8 = sp.tile([128, 8], F32, tag=f"m8{tag}", name=f"m8{tag}_{r}_{id(vals)}")
        nc.vector.max(out=m8[:], in_=vals)
        nc.vector.max_index(out=out_ids[:, r * 8:(r + 1) * 8], in_max=m8[:],
                            in_values=vals)
        if r < 2:
            nc.vector.match_replace(out=vals, in_to_replace=m8[:], in_values=vals,
                                    imm_value=NEG)


def _build_l1(repeat=1):
    nc = bacc.Bacc("TRN2", target_bir_lowering=False, debug=False,
                   num_devices=NCORES)
    xyzT_d = nc.dram_tensor("xyzT", [11, N], F32R, kind="ExternalInput").ap()
    qT_d = nc.dram_tensor("qT", [11, NQ], F32R, kind="ExternalInput").ap()
    ids_d = nc.dram_tensor("ids", [NQ, 24], U16, kind="ExternalOutput").ap()
    with tile.TileContext(nc) as tc:
        with (
            tc.tile_pool(name="tabs", bufs=1) as tabs,
            tc.tile_pool(name="psum", bufs=1, space="PSUM") as pp,
            tc.tile_pool(name="workbig", bufs=3) as wb,
            tc.tile_pool(name="work", bufs=6) as wp,
            tc.tile_pool(name="small", bufs=8) as sp,
        ):
            xyzT_sb = tabs.tile([11, N], F32R)
            qT_sb = tabs.tile([11, NQ], F32R)
            warm = tabs.tile([11, 512], F32)
            ids_all = tabs.tile([128, NBLK * 24], U16)
            nc.vector.memset(warm[:], 0.0)
            nc.sync.dma_start(out=xyzT_sb[:], in_=xyzT_d[:])
            nc.sync.dma_start(out=qT_sb[:], in_=qT_d[:])
            # ramp the PE to full clock while the inputs stream in
            pswarm = pp.tile([128, 2048], F32, tag="psA", name="pswarm")
            for n in range(10):
                nc.tensor.matmul(pswarm[:, 0:512], warm[:, 0:128], warm[:],
                                 start=True, stop=True)
            c16s = {}
            nblk = repeat * NBLK
            for i in range(nblk):
                ib = i % NBLK
                lhsT = qT_sb[:, ib * 128:(ib + 1) * 128]
                psA = pp.tile([128, 2048], F32, tag="psA", name=f"psA_{i}")
                psB = pp.tile([128, 2048], F32, tag="psB", name=f"psB_{i}")
                for k in range(4):
                    nc.tensor.matmul(psA[:, k * 512:(k + 1) * 512], lhsT,
                                     xyzT_sb[:, k * 512:(k + 1) * 512],
                                     start=True, stop=True)
                for k in range(4):
                    nc.tensor.matmul(psB[:, k * 512:(k + 1) * 512], lhsT,
                                     xyzT_sb[:, (4 + k) * 512:(5 + k) * 512],
                                     start=True, stop=True)
                # HW: Pool can't read PSUM and ops take at most one PSUM
                # operand, so Act (otherwise idle) evacuates PSUM to SBUF and
                # the pair-max tree runs in SBUF. Pairings keep chunk=pt%256.
                cpB = wb.tile([128, 2048], F32, tag="cpB", name=f"cpB_{i}")
                nc.scalar.activation(cpB[:], psB[:], AF.Copy)
                s1 = wb.tile([128, 2048], F32, tag="s1", name=f"s1_{i}")
                nc.vector.tensor_tensor(s1[:], psA[:], cpB[:], op=OP.max)
                s2 = wp.tile([128, 1024], F32, tag="s2", name=f"s2_{i}")
                nc.vector.tensor_tensor(s2[:], s1[:, 0:1024], s1[:, 1024:2048],
                                        op=OP.max)
                s3 = wp.tile([128, 512], F32, tag="s3", name=f"s3_{i}")
                nc.vector.tensor_tensor(s3[:], s2[:, 0:512], s2[:, 512:1024],
                                        op=OP.max)
                c16 = wp.tile([128, NCH], F32, tag="c16", name=f"c16_{i}")
                nc.vector.tensor_tensor(c16[:], s3[:, 0:256], s3[:, 256:512],
                                        op=OP.max)
                c16s[i] = c16
                # software pipelining: selection rounds lag the reduce by one
                # block so the DVE->Pool->DVE cycle spans two periods
                if i >= 1:
                    j = i - 1
                    _rounds(nc, sp, c16s.pop(j)[:],
                            ids_all[:, (j % NBLK) * 24:(j % NBLK + 1) * 24], "a")
            j = nblk - 1
            _rounds(nc, sp, c16s.pop(j)[:],
                    ids_all[:, (j % NBLK) * 24:(j % NBLK + 1) * 24], "a")
            nc.sync.dma_start(
                out=ids_d[:].rearrange("(i p) s -> p i s", p=128),
                in_=ids_all[:].rearrange("p (i s) -> p i s", s=24))
    nc.compile()
    return nc


def _build_l2a(repeat=1):
    nc = bacc.Bacc("TRN2", target_bir_lowering=False, debug=False,
                   num_devices=NCORES)
    g_d = nc.dram_tensor("g", [NQ, 3 * W], F32, kind="ExternalInput").ap()
    nq_d = nc.dram_tensor("nq", [NQ, 3], F32, kind="ExternalInput").ap()
    loc_d = nc.dram_tensor("loc", [NQ, 24], U16, kind="ExternalOutput").ap()
    with tile.TileContext(nc) as tc:
        with (
            tc.tile_pool(name="tabs", bufs=1) as tabs,
            tc.tile_pool(name="work", bufs=6) as wp,
            tc.tile_pool(name="small", bufs=6) as sp,
        ):
            nq_all = tabs.tile([128, NBLK * 3], F32)
            loc_all = tabs.tile([128, NBLK * 24], U16)
            zz = tabs.tile([128, W], F32)
            nc.vector.memset(zz[:], 0.0)
            gt0 = wp.tile([128, 3 * W], F32, tag="gt", name="gt_0")
            nc.sync.dma_start(out=gt0[:], in_=g_d[0:128, :])
            nc.sync.dma_start(
                out=nq_all[:].rearrange("p (i c) -> p i c", c=3),
                in_=nq_d[:].rearrange("(i p) c -> p i c", p=128))
            for i in range(repeat * NBLK):
                ib = i % NBLK
                sl = slice(ib * 128, (ib + 1) * 128)
                if i == 0:
                    gt = gt0
                else:
                    gt = wp.tile([128, 3 * W], F32, tag="gt", name=f"gt_{i}")
                    nc.sync.dma_start(out=gt[:], in_=g_d[sl, :])
                nsq = wp.tile([128, 3, W], F32, tag="nsq", name=f"nsq_{i}")
                for c in range(3):
                    nc.scalar.activation(nsq[:, c, :], gt[:, c * W:(c + 1) * W],
                                         AF.Square,
                                         bias=nq_all[:, ib * 3 + c:ib * 3 + c + 1],
                                         scale=1.0)
                # nd = ((0-sq0)-sq1)-sq2, bit-identical to reference order
                nd = wp.tile([128, W], F32, tag="nd", name=f"nd_{i}")
                nc.gpsimd.tensor_tensor(nd[:], zz[:], nsq[:, 0, :], op=OP.subtract)
                nc.gpsimd.tensor_tensor(nd[:], nd[:], nsq[:, 1, :], op=OP.subtract)
                nc.gpsimd.tensor_tensor(nd[:], nd[:], nsq[:, 2, :], op=OP.subtract)
                _rounds(nc, sp, nd[:], loc_all[:, ib * 24:(ib + 1) * 24], "b")
            nc.sync.dma_start(
                out=loc_d[:].rearrange("(i p) s -> p i s", p=128),
                in_=loc_all[:].rearrange("p (i s) -> p i s", s=24))
    nc.compile()
    return nc


def _build_l2b(repeat=1):
    nc = bacc.Bacc("TRN2", target_bir_lowering=False, debug=False,
                   num_devices=NCORES)
    gx_d = nc.dram_tensor("gx12", [12, NQ * 8], F32R, kind="ExternalInput").ap()
    w12_d = nc.dram_tensor("w12b", [12, 128], F32R, kind="ExternalInput").ap()
    w2_d = nc.dram_tensor("w2b", [128, 128], F32R, kind="ExternalInput").ap()
    w3_d = nc.dram_tensor("w3b", [128, 128], F32R, kind="ExternalInput").ap()
    eye_d = nc.dram_tensor("eye", [128, 128], F32, kind="ExternalInput").ap()
    out_d = nc.dram_tensor("out", [NQ, C], F32, kind="ExternalOutput").ap()
    with tile.TileContext(nc) as tc:
        with (
            tc.tile_pool(name="tabs", bufs=1) as tabs,
            tc.tile_pool(name="psum", bufs=1, space="PSUM") as pp,
            tc.tile_pool(name="psumT", bufs=2, space="PSUM") as ppt,
            tc.tile_pool(name="work", bufs=4) as wp,
            tc.tile_pool(name="small", bufs=4) as sp,
        ):
            w12_sb = tabs.tile([12, 128], F32R)
            w2_sb = tabs.tile([128, 128], F32R)
            w3_sb = tabs.tile([128, 128], F32R)
            eye_sb = tabs.tile([128, 128], F32)
            gx_sb = tabs.tile([12, NQ * 8], F32R)
            fin_all = tabs.tile([128, NBLK * 64], F32)
            for sb, dd in ((w12_sb, w12_d), (w2_sb, w2_d),
                           (w3_sb, w3_d), (eye_sb, eye_d), (gx_sb, gx_d)):
                nc.sync.dma_start(out=sb[:], in_=dd[:])
            warm = tabs.tile([12, 512], F32)
            nc.vector.memset(warm[:], 0.0)
            pswarm = pp.tile([128, 512], F32, tag="ps10", name="pswarm")
            for n in range(8):
                nc.tensor.matmul(pswarm[:], warm[:, 0:128], warm[:],
                                 start=True, stop=True)
            mxs = {}

            def _tail(j):
                jb = j % NBLK
                pst = ppt.tile([128, 128], F32, tag="pst", name=f"pst_{j}")
                nc.tensor.transpose(pst[:], mxs.pop(j)[:], eye_sb[:])
                mxT = sp.tile([128, 128], F32, tag="mxT", name=f"mxT_{j}")
                nc.scalar.activation(mxT[:], pst[:], AF.Copy)
                nc.vector.tensor_tensor(fin_all[:, jb * 64:(jb + 1) * 64],
                                        mxT[:, 0:64], mxT[:, 64:128], op=OP.max)

            for i in range(repeat * NBLK):
                ib = i % NBLK
                mx = sp.tile([128, 128], F32, tag="mx", name=f"mx_{i}")
                cs = [slice(ib * 1024 + t * 512, ib * 1024 + (t + 1) * 512)
                      for t in range(2)]
                # interleave the two independent halves so their chains run
                # concurrently on different engines
                ps1 = [pp.tile([128, 512], F32, tag=f"ps1{t}", name=f"ps1_{i}_{t}")
                       for t in range(2)]
                for t in range(2):
                    nc.tensor.matmul(ps1[t][:], w12_sb[:], gx_sb[:, cs[t]],
                                     start=True, stop=True)
                h1 = [wp.tile([128, 512], F32R, tag=f"h1{t}", name=f"h1_{i}_{t}")
                      for t in range(2)]
                nc.scalar.activation(h1[0][:], ps1[0][:], AF.Relu)
                nc.vector.tensor_scalar(h1[1][:], ps1[1][:], 0.0, scalar2=None,
                                        op0=OP.max)
                ps2 = [pp.tile([128, 512], F32, tag=f"ps2{t}", name=f"ps2_{i}_{t}")
                       for t in range(2)]
                for t in range(2):
                    nc.tensor.matmul(ps2[t][:], w2_sb[:], h1[t][:],
                                     start=True, stop=True)
                h2 = [wp.tile([128, 512], F32R, tag=f"h2{t}", name=f"h2_{i}_{t}")
                      for t in range(2)]
                nc.scalar.activation(h2[0][:], ps2[0][:], AF.Relu)
                nc.scalar.activation(h2[1][:], ps2[1][:], AF.Relu)
                ps3 = [pp.tile([128, 512], F32, tag=f"ps3{t}", name=f"ps3_{i}_{t}")
                       for t in range(2)]
                for t in range(2):
                    nc.tensor.matmul(ps3[t][:], w3_sb[:], h2[t][:],
                                     start=True, stop=True)
                # neighbor max-pool: single-PSUM-input TensorReduce on DVE
                for t in range(2):
                    nc.vector.tensor_reduce(
                        mx[:, t * 64:(t + 1) * 64],
                        ps3[t][:].rearrange("m (q p) -> m q p", p=8),
                        axis=AX.X, op=OP.max)
                mxs[i] = mx
                # transpose+final-max lag one block so the PE stream is never
                # head-of-line blocked on the current block's pool tree
                if i >= 1:
                    _tail(i - 1)
            _tail(repeat * NBLK - 1)
            nc.sync.dma_start(
                out=out_d[:].rearrange("(i p) c -> p i c", p=128),
                in_=fin_all[:].rearrange("p (i c) -> p i c", c=64))
    nc.compile()
    return nc


class _Executor:
    """Cached multi-core PJRT executor for one prebuilt Bass program."""

    def __init__(self, nc):
        install_neuronx_cc_hook()
        self.nc = nc
        part_name = nc.partition_id_tensor.name if nc.partition_id_tensor else None
        in_names, out_names, out_avals, zero_outs = [], [], [], []
        for alloc in nc.m.functions[0].allocations:
            if not isinstance(alloc, mybir.MemoryLocationSet):
                continue
            name = alloc.memorylocations[0].name
            if alloc.kind == "ExternalInput":
                if name != part_name:
                    in_names.append(name)
            elif alloc.kind == "ExternalOutput":
                shape = tuple(alloc.tensor_shape)
                dtype = mybir.dt.np(alloc.dtype)
                out_names.append(name)
                out_avals.append(jax.core.ShapedArray(shape, dtype))
                zero_outs.append(_np.zeros(shape, dtype))
        self.in_names, self.out_names = in_names, out_names
        self.out_avals, self.zero_outs = out_avals, zero_outs
        n_params = len(in_names)
        all_names = in_names + out_names
        if part_name is not None:
            all_names = all_names + [part_name]

        def _body(*args):
            operands = list(args)
            if part_name is not None:
                operands.append(bass2jax.partition_id_tensor())
            return tuple(_bass_exec_p.bind(
                *operands,
                out_avals=tuple(out_avals),
                in_names=tuple(all_names),
                out_names=tuple(out_names),
                lowering_input_output_aliases=(),
                sim_require_finite=True,
                sim_require_nnan=True,
                nc=nc,
            ))

        devices = jax.devices()[:NCORES]
        mesh = Mesh(_np.asarray(devices), ("core",))
        n_outs = len(out_names)
        self._fn = jax.jit(
            shard_map(_body, mesh=mesh,
                      in_specs=(PartitionSpec("core"),) * (n_params + n_outs),
                      out_specs=(PartitionSpec("core"),) * n_outs,
                      check_rep=False),
            donate_argnums=tuple(range(n_params, n_params + n_outs)),
            keep_unused=True,
        )

    def prepare(self, in_maps):
        n = NCORES
        return [
            _np.concatenate([_np.asarray(in_maps[c][name]) for c in range(n)], axis=0)
            for name in self.in_names
        ]

    def run_prepared(self, concat_in):
        n = NCORES
        concat_zeros = [_np.zeros((n * z.shape[0], *z.shape[1:]), z.dtype)
                        for z in self.zero_outs]
        return self._fn(*concat_in, *concat_zeros)

    def __call__(self, in_maps):
        n = NCORES
        outs = self.run_prepared(self.prepare(in_maps))
        outs = [_np.asarray(o) for o in outs]
        return [
            {name: outs[i].reshape(n, *self.out_avals[i].shape)[c]
             for i, name in enumerate(self.out_names)}
            for c in range(n)
        ]


def _get_progs():
    if "l1" not in _progs:
        _progs["l1"] = _Executor(_build_l1())
        _progs["l2a"] = _Executor(_build_l2a())
        _progs["l2b"] = _Executor(_build_l2b())
    return _progs["l1"], _progs["l2a"], _progs["l2b"]


def kernel(xyz, w1, w2, w3, k):
    xyz = np.asarray(xyz, dtype=np.float32)
    w1 = np.asarray(w1, dtype=np.float32)
    w2 = np.asarray(w2, dtype=np.float32)
    w3 = np.asarray(w3, dtype=np.float32)
    assert int(k) == K and xyz.shape == (B, N, 3)
    l1, l2a, l2b = _get_progs()
    cores = list(range(NCORES))

    # ---- L1: coarse chunk selection -------------------------------------
    # hi/lo fp32r decomposition restores ~fp32 scoring accuracy on the PE:
    # score = qh.vh + qh.vl + ql.vh - sqh - sql  (v = 2x, sq = |x|^2)
    xyzT_b = []
    for b in range(B):
        X = xyz[b]
        sq = (X[:, 0] ** 2 + X[:, 1] ** 2 + X[:, 2] ** 2).astype(np.float32)
        v = (2.0 * X.T).astype(np.float32)                   # (3, N)
        vh, vl = _hilo(v)
        sqh, sql = _hilo(sq)
        xyzT_b.append(np.concatenate(
            [vh, vl, vh, sqh[None, :], sql[None, :]]).astype(np.float32))
    in1 = []
    for c in cores:
        b, h = c // 2, c % 2
        Q = xyz[b, h * NQ:(h + 1) * NQ]
        qh, ql = _hilo(Q.T.astype(np.float32))               # (3, NQ)
        ones = -np.ones((1, NQ), np.float32)
        qT = np.concatenate([qh, qh, ql, ones, ones]).astype(np.float32)
        in1.append({"xyzT": xyzT_b[b], "qT": qT})
    r1 = l1(in1)

    # ---- host glue: superset gather (chunk c members = c + 256*j) -------
    sup = []   # per-core (NQ, W) global candidate ids
    in2 = []
    for c in cores:
        b, h = c // 2, c % 2
        ids = r1[c]["ids"][:, :NSEL].astype(np.int64)          # (NQ, 20)
        s = (ids[:, :, None] + (np.arange(CH) * NCH)[None, None, :]).reshape(NQ, W)
        sup.append(s)
        g = xyz[b][s]                                          # (NQ, W, 3)
        g3 = np.ascontiguousarray(g.transpose(0, 2, 1)).reshape(NQ, 3 * W)
        nq3 = -np.ascontiguousarray(xyz[b, h * NQ:(h + 1) * NQ])
        in2.append({"g": g3.astype(np.float32), "nq": nq3.astype(np.float32)})
    r2 = l2a(in2)

    # ---- host glue: final-16 gather ------------------------------------
    w1blkT = np.zeros((6, 128), np.float32)
    w1blkT[0:3, 0:64] = w1.T
    w1blkT[3:6, 64:128] = w1.T
    w2blkT = np.zeros((128, 128), np.float32)
    w2blkT[0:64, 0:64] = w2.T
    w2blkT[64:128, 64:128] = w2.T
    w3blkT = np.zeros((128, 128), np.float32)
    w3blkT[0:64, 0:64] = w3.T
    w3blkT[64:128, 64:128] = w3.T
    eye = np.eye(128, dtype=np.float32)
    in3 = []
    for c in cores:
        b, h = c // 2, c % 2
        loc = r2[c]["loc"].astype(np.int64)            # (NQ, 24)
        glob = np.take_along_axis(sup[c], loc[:, 1:KK], axis=1)  # (NQ, 16)
        g16 = xyz[b][glob]                                     # (NQ, 16, 3)
        gA, gB = g16[:, 0::2, :], g16[:, 1::2, :]
        g6 = np.concatenate([gA, gB], axis=2)                  # (NQ, 8, 6)
        g6 = np.ascontiguousarray(g6.transpose(2, 0, 1)).reshape(6, NQ * 8)
        q = xyz[b, h * NQ:(h + 1) * NQ]
        xq6 = np.repeat(np.concatenate([q, q], axis=1)[:, None, :], 8, axis=1)
        xq6 = np.ascontiguousarray(xq6.transpose(2, 0, 1)).reshape(6, NQ * 8)
        in3.append({"gx12": _r32r(np.concatenate([g6, xq6], axis=0)),
                    "w12b": _r32r(np.concatenate([w1blkT, -w1blkT], axis=0)),
                    "w2b": _r32r(w2blkT), "w3b": _r32r(w3blkT), "eye": eye})
    r3 = l2b(in3)

    out = np.zeros((B, C, N), np.float32)
    for c in cores:
        b, h = c // 2, c % 2
        out[b, :, h * NQ:(h + 1) * NQ] = r3[c]["out"].T
    return out


# revision 49
# speedup vs baseline: 1.0087x; 1.0086x over previous
"""kNN (k=16) + grouped 3->64->64->64 MLP + neighbor max-pool on 8 TRN2 cores.

Pipeline (device does all O(N^2) compute, selection, and MLP flops):
  L1 : S[q,j] = 2<xq,xj> - |xj|^2 on PE as fp32r matmuls (1 cyc/col) with a
       hi/lo fp32r operand decomposition (contraction 11) restoring ~fp32
       accuracy; Act evacuates PSUM to SBUF (Pool cannot read PSUM and only
       DVE can compute max on HW); chunk-16 max via a DVE pairwise max tree
       over contiguous slabs (chunk c = points {c + 256*j}); top-24 chunk
       ids via max8/max_index/match_replace rounds on DVE, software-
       pipelined one block behind the reduce.
  host: gather the 20*16=320 candidate coords per query (index routing).
  L2A: exact squared dists in reference fp32 arithmetic on the 320-wide
       compacted domain (Act squares, Pool subtract chain); exact top-17
       (slot 0 = self) via DVE rounds -> local indices; per-block constants
       preloaded in one DMA, ids/loc written out in one batched DMA.
  L2B: relative coords via matmul-folded subtract, packed 2-point 3-layer
       MLP on PE (fp32r, host-prerounded weights/inputs), neighbor max-pool
       as single-PSUM-input DVE TensorReduce, PE transpose lagged one block,
       final pair max.

Sharding: core c handles batch c//2, query half c%2 (2048 queries each).
"""
import sys
import numpy as np

sys.path.insert(0, "/opt/trn_rl_repo")

import jax
import numpy as _np
from jax.sharding import Mesh, PartitionSpec
from jax.experimental.shard_map import shard_map

import concourse.bacc as bacc
import concourse.mybir as mybir
import concourse.tile as tile
from concourse import bass2jax
from concourse.bass2jax import _bass_exec_p, install_neuronx_cc_hook

F32 = mybir.dt.float32
F32R = mybir.dt.float32r
U16 = mybir.dt.uint16
AX = mybir.AxisListType
OP = mybir.AluOpType
AF = mybir.ActivationFunctionType

def _r32r(a):
    """Round fp32 to fp32r (11 explicit mantissa bits, round-half-up @bit 12)."""
    b = np.ascontiguousarray(a, dtype=np.float32).view(np.uint32)
    b = ((b.astype(np.uint64) + 0x800) & 0xFFFFF000).astype(np.uint32)
    return b.view(np.float32)


def _hilo(a):
    h = _r32r(a)
    return h, _r32r((a.astype(np.float32) - h).astype(np.float32))


B, N, C, K = 4, 4096, 64, 16
KK = K + 1              # 17
CH = 16                 # points per chunk
NCH = N // CH           # 256 chunks; member j of chunk c = point c + 256*j
NSEL = 20               # chunks kept per query (17 guarantee + 3 tie slack)
W = NSEL * CH           # 320 candidate superset per query
NQ = 2048               # queries per core
NBLK = NQ // 128        # 16
NEG = -1.0e30
NCORES = 8
B1P = 576               # Pool share of the B-half first max-tree level

_progs = {}


def _rounds(nc, sp, vals, out_ids, tag):
    """3x (max8 -> max_index -> match_replace) producing 24 ids, mutating vals."""
    for r in range(3):
        m8 = sp.tile([128, 8], F32, tag=f"m8{tag}", name=f"m8{tag}_{r}_{id(vals)}")
        nc.vector.max(out=m8[:], in_=vals)
        nc.vector.max_index(out=out_ids[:, r * 8:(r + 1) * 8], in_max=m8[:],
                            in_values=vals)
        if r < 2:
            nc.vector.match_replace(out=vals, in_to_replace=m8[:], in_values=vals,
                                    imm_value=NEG)


def _build_l1(repeat=1):
    nc = bacc.Bacc("TRN2", target_bir_lowering=False, debug=False,
                   num_devices=NCORES)
    xyzT_d = nc.dram_tensor("xyzT", [11, N], F32R, kind="ExternalInput").ap()
    qT_d = nc.dram_tensor("qT", [11, NQ], F32R, kind="ExternalInput").ap()
    ids_d = nc.dram_tensor("ids", [NQ, 24], U16, kind="ExternalOutput").ap()
    with tile.TileContext(nc) as tc:
        with (
            tc.tile_pool(name="tabs", bufs=1) as tabs,
            tc.tile_pool(name="psum", bufs=1, space="PSUM") as pp,
            tc.tile_pool(name="workbig", bufs=3) as wb,
            tc.tile_pool(name="work", bufs=6) as wp,
            tc.tile_pool(name="small", bufs=8) as sp,
        ):
            xyzT_sb = tabs.tile([11, N], F32R)
            qT_sb = tabs.tile([11, NQ], F32R)
            warm = tabs.tile([11, 512], F32)
            ids_all = tabs.tile([128, NBLK * 24], U16)
            nc.vector.memset(warm[:], 0.0)
            nc.sync.dma_start(out=xyzT_sb[:], in_=xyzT_d[:])
            nc.sync.dma_start(out=qT_sb[:], in_=qT_d[:])
            # ramp the PE to full clock while the inputs stream in
            pswarm = pp.tile([128, 2048], F32, tag="psA", name="pswarm")
            for n in range(10):
                nc.tensor.matmul(pswarm[:, 0:512], warm[:, 0:128], warm[:],
                                 start=True, stop=True)
            c16s = {}
            nblk = repeat * NBLK
            for i in range(nblk):
                ib = i % NBLK
                lhsT = qT_sb[:, ib * 128:(ib + 1) * 128]
                psA = pp.tile([128, 2048], F32, tag="psA", name=f"psA_{i}")
                psB = pp.tile([128, 2048], F32, tag="psB", name=f"psB_{i}")
                for k in range(4):
                    nc.tensor.matmul(psA[:, k * 512:(k + 1) * 512], lhsT,
                                     xyzT_sb[:, k * 512:(k + 1) * 512],
                                     start=True, stop=True)
                for k in range(4):
                    nc.tensor.matmul(psB[:, k * 512:(k + 1) * 512], lhsT,
                                     xyzT_sb[:, (4 + k) * 512:(5 + k) * 512],
                                     start=True, stop=True)
                # HW: Pool can't read PSUM and ops take at most one PSUM
                # operand, so Act (otherwise idle) evacuates PSUM to SBUF and
                # the pair-max tree runs in SBUF. Pairings keep chunk=pt%256.
                cpB = wb.tile([128, 2048], F32, tag="cpB", name=f"cpB_{i}")
                nc.scalar.activation(cpB[:], psB[:], AF.Copy)
                s1 = wb.tile([128, 2048], F32, tag="s1", name=f"s1_{i}")
                nc.vector.tensor_tensor(s1[:], psA[:], cpB[:], op=OP.max)
                s2 = wp.tile([128, 1024], F32, tag="s2", name=f"s2_{i}")
                nc.vector.tensor_tensor(s2[:], s1[:, 0:1024], s1[:, 1024:2048],
                                        op=OP.max)
                s3 = wp.tile([128, 512], F32, tag="s3", name=f"s3_{i}")
                nc.vector.tensor_tensor(s3[:], s2[:, 0:512], s2[:, 512:1024],
                                        op=OP.max)
                c16 = wp.tile([128, NCH], F32, tag="c16", name=f"c16_{i}")
                nc.vector.tensor_tensor(c16[:], s3[:, 0:256], s3[:, 256:512],
                                        op=OP.max)
                c16s[i] = c16
                # software pipelining: selection rounds lag the reduce by one
                # block so the DVE->Pool->DVE cycle spans two periods
                if i >= 1:
                    j = i - 1
                    _rounds(nc, sp, c16s.pop(j)[:],
                            ids_all[:, (j % NBLK) * 24:(j % NBLK + 1) * 24], "a")
                    if j == NBLK // 2 - 1:
                        nc.sync.dma_start(
                            out=ids_d[0:NQ // 2].rearrange("(i p) s -> p i s", p=128),
                            in_=ids_all[:, 0:NBLK // 2 * 24].rearrange(
                                "p (i s) -> p i s", s=24))
            j = nblk - 1
            _rounds(nc, sp, c16s.pop(j)[:],
                    ids_all[:, (j % NBLK) * 24:(j % NBLK + 1) * 24], "a")
            nc.sync.dma_start(
                out=ids_d[NQ // 2:NQ].rearrange("(i p) s -> p i s", p=128),
                in_=ids_all[:, NBLK // 2 * 24:].rearrange("p (i s) -> p i s", s=24))
    nc.compile()
    return nc


def _build_l2a(repeat=1):
    nc = bacc.Bacc("TRN2", target_bir_lowering=False, debug=False,
                   num_devices=NCORES)
    g_d = nc.dram_tensor("g", [NQ, 3 * W], F32, kind="ExternalInput").ap()
    nq_d = nc.dram_tensor("nq", [NQ, 3], F32, kind="ExternalInput").ap()
    loc_d = nc.dram_tensor("loc", [NQ, 24], U16, kind="ExternalOutput").ap()
    with tile.TileContext(nc) as tc:
        with (
            tc.tile_pool(name="tabs", bufs=1) as tabs,
            tc.tile_pool(name="work", bufs=6) as wp,
            tc.tile_pool(name="small", bufs=6) as sp,
        ):
            nq_all = tabs.tile([128, NBLK * 3], F32)
            loc_all = tabs.tile([128, NBLK * 24], U16)
            zz = tabs.tile([128, W], F32)
            nc.vector.memset(zz[:], 0.0)
            gt0 = wp.tile([128, 3 * W], F32, tag="gt", name="gt_0")
            nc.sync.dma_start(out=gt0[:], in_=g_d[0:128, :])
            nc.sync.dma_start(
                out=nq_all[:].rearrange("p (i c) -> p i c", c=3),
                in_=nq_d[:].rearrange("(i p) c -> p i c", p=128))
            for i in range(repeat * NBLK):
                ib = i % NBLK
                sl = slice(ib * 128, (ib + 1) * 128)
                if i == 0:
                    gt = gt0
                else:
                    gt = wp.tile([128, 3 * W], F32, tag="gt", name=f"gt_{i}")
                    nc.sync.dma_start(out=gt[:], in_=g_d[sl, :])
                nsq = wp.tile([128, 3, W], F32, tag="nsq", name=f"nsq_{i}")
                for c in range(3):
                    nc.scalar.activation(nsq[:, c, :], gt[:, c * W:(c + 1) * W],
                                         AF.Square,
                                         bias=nq_all[:, ib * 3 + c:ib * 3 + c + 1],
                                         scale=1.0)
                # nd = ((0-sq0)-sq1)-sq2, bit-identical to reference order
                nd = wp.tile([128, W], F32, tag="nd", name=f"nd_{i}")
                nc.gpsimd.tensor_tensor(nd[:], zz[:], nsq[:, 0, :], op=OP.subtract)
                nc.gpsimd.tensor_tensor(nd[:], nd[:], nsq[:, 1, :], op=OP.subtract)
                nc.gpsimd.tensor_tensor(nd[:], nd[:], nsq[:, 2, :], op=OP.subtract)
                _rounds(nc, sp, nd[:], loc_all[:, ib * 24:(ib + 1) * 24], "b")
                if ib == NBLK // 2 - 1:
                    nc.sync.dma_start(
                        out=loc_d[0:NQ // 2].rearrange("(i p) s -> p i s", p=128),
                        in_=loc_all[:, 0:NBLK // 2 * 24].rearrange(
                            "p (i s) -> p i s", s=24))
            nc.sync.dma_start(
                out=loc_d[NQ // 2:NQ].rearrange("(i p) s -> p i s", p=128),
                in_=loc_all[:, NBLK // 2 * 24:].rearrange("p (i s) -> p i s", s=24))
    nc.compile()
    return nc


def _build_l2b(repeat=1):
    nc = bacc.Bacc("TRN2", target_bir_lowering=False, debug=False,
                   num_devices=NCORES)
    gx_d = nc.dram_tensor("gx12", [12, NQ * 8], F32R, kind="ExternalInput").ap()
    w12_d = nc.dram_tensor("w12b", [12, 128], F32R, kind="ExternalInput").ap()
    w2_d = nc.dram_tensor("w2b", [128, 128], F32R, kind="ExternalInput").ap()
    w3_d = nc.dram_tensor("w3b", [128, 128], F32R, kind="ExternalInput").ap()
    eye_d = nc.dram_tensor("eye", [128, 128], F32, kind="ExternalInput").ap()
    out_d = nc.dram_tensor("out", [NQ, C], F32, kind="ExternalOutput").ap()
    with tile.TileContext(nc) as tc:
        with (
            tc.tile_pool(name="tabs", bufs=1) as tabs,
            tc.tile_pool(name="psum", bufs=1, space="PSUM") as pp,
            tc.tile_pool(name="psumT", bufs=2, space="PSUM") as ppt,
            tc.tile_pool(name="work", bufs=4) as wp,
            tc.tile_pool(name="small", bufs=4) as sp,
        ):
            w12_sb = tabs.tile([12, 128], F32R)
            w2_sb = tabs.tile([128, 128], F32R)
            w3_sb = tabs.tile([128, 128], F32R)
            eye_sb = tabs.tile([128, 128], F32)
            gx_sb = tabs.tile([12, NQ * 8], F32R)
            fin_all = tabs.tile([128, NBLK * 64], F32)
            for sb, dd in ((w12_sb, w12_d), (w2_sb, w2_d),
                           (w3_sb, w3_d), (eye_sb, eye_d), (gx_sb, gx_d)):
                nc.sync.dma_start(out=sb[:], in_=dd[:])
            warm = tabs.tile([12, 512], F32)
            nc.vector.memset(warm[:], 0.0)
            pswarm = pp.tile([128, 512], F32, tag="ps10", name="pswarm")
            for n in range(8):
                nc.tensor.matmul(pswarm[:], warm[:, 0:128], warm[:],
                                 start=True, stop=True)
            mxs = {}

            def _tail(j):
                jb = j % NBLK
                pst = ppt.tile([128, 128], F32, tag="pst", name=f"pst_{j}")
                nc.tensor.transpose(pst[:], mxs.pop(j)[:], eye_sb[:])
                mxT = sp.tile([128, 128], F32, tag="mxT", name=f"mxT_{j}")
                nc.scalar.activation(mxT[:], pst[:], AF.Copy)
                nc.vector.tensor_tensor(fin_all[:, jb * 64:(jb + 1) * 64],
                                        mxT[:, 0:64], mxT[:, 64:128], op=OP.max)

            for i in range(repeat * NBLK):
                ib = i % NBLK
                mx = sp.tile([128, 128], F32, tag="mx", name=f"mx_{i}")
                cs = [slice(ib * 1024 + t * 512, ib * 1024 + (t + 1) * 512)
                      for t in range(2)]
                # interleave the two independent halves so their chains run
                # concurrently on different engines
                ps1 = [pp.tile([128, 512], F32, tag=f"ps1{t}", name=f"ps1_{i}_{t}")
                       for t in range(2)]
                for t in range(2):
                    nc.tensor.matmul(ps1[t][:], w12_sb[:], gx_sb[:, cs[t]],
                                     start=True, stop=True)
                h1 = [wp.tile([128, 512], F32R, tag=f"h1{t}", name=f"h1_{i}_{t}")
                      for t in range(2)]
                nc.scalar.activation(h1[0][:], ps1[0][:], AF.Relu)
                nc.vector.tensor_scalar(h1[1][:], ps1[1][:], 0.0, scalar2=None,
                                        op0=OP.max)
                ps2 = [pp.tile([128, 512], F32, tag=f"ps2{t}", name=f"ps2_{i}_{t}")
                       for t in range(2)]
                for t in range(2):
                    nc.tensor.matmul(ps2[t][:], w2_sb[:], h1[t][:],
                                     start=True, stop=True)
                h2 = [wp.tile([128, 512], F32R, tag=f"h2{t}", name=f"h2_{i}_{t}")
                      for t in range(2)]
                nc.scalar.activation(h2[0][:], ps2[0][:], AF.Relu)
                nc.scalar.activation(h2[1][:], ps2[1][:], AF.Relu)
                ps3 = [pp.tile([128, 512], F32, tag=f"ps3{t}", name=f"ps3_{i}_{t}")
                       for t in range(2)]
                for t in range(2):
                    nc.tensor.matmul(ps3[t][:], w3_sb[:], h2[t][:],
                                     start=True, stop=True)
                # neighbor max-pool: single-PSUM-input TensorReduce on DVE
                for t in range(2):
                    nc.vector.tensor_reduce(
                        mx[:, t * 64:(t + 1) * 64],
                        ps3[t][:].rearrange("m (q p) -> m q p", p=8),
                        axis=AX.X, op=OP.max)
                mxs[i] = mx
                # transpose+final-max lag one block so the PE stream is never
                # head-of-line blocked on the current block's pool tree
                if i >= 1:
                    _tail(i - 1)
                    if i - 1 == NBLK // 2 - 1:
                        nc.sync.dma_start(
                            out=out_d[0:NQ // 2].rearrange("(i p) c -> p i c", p=128),
                            in_=fin_all[:, 0:NBLK // 2 * 64].rearrange(
                                "p (i c) -> p i c", c=64))
            _tail(repeat * NBLK - 1)
            nc.sync.dma_start(
                out=out_d[NQ // 2:NQ].rearrange("(i p) c -> p i c", p=128),
                in_=fin_all[:, NBLK // 2 * 64:].rearrange("p (i c) -> p i c", c=64))
    nc.compile()
    return nc


class _Executor:
    """Cached multi-core PJRT executor for one prebuilt Bass program."""

    def __init__(self, nc):
        install_neuronx_cc_hook()
        self.nc = nc
        part_name = nc.partition_id_tensor.name if nc.partition_id_tensor else None
        in_names, out_names, out_avals, zero_outs = [], [], [], []
        for alloc in nc.m.functions[0].allocations:
            if not isinstance(alloc, mybir.MemoryLocationSet):
                continue
            name = alloc.memorylocations[0].name
            if alloc.kind == "ExternalInput":
                if name != part_name:
                    in_names.append(name)
            elif alloc.kind == "ExternalOutput":
                shape = tuple(alloc.tensor_shape)
                dtype = mybir.dt.np(alloc.dtype)
                out_names.append(name)
                out_avals.append(jax.core.ShapedArray(shape, dtype))
                zero_outs.append(_np.zeros(shape, dtype))
        self.in_names, self.out_names = in_names, out_names
        self.out_avals, self.zero_outs = out_avals, zero_outs
        n_params = len(in_names)
        all_names = in_names + out_names
        if part_name is not None:
            all_names = all_names + [part_name]

        def _body(*args):
            operands = list(args)
            if part_name is not None:
                operands.append(bass2jax.partition_id_tensor())
            return tuple(_bass_exec_p.bind(
                *operands,
                out_avals=tuple(out_avals),
                in_names=tuple(all_names),
                out_names=tuple(out_names),
                lowering_input_output_aliases=(),
                sim_require_finite=True,
                sim_require_nnan=True,
                nc=nc,
            ))

        devices = jax.devices()[:NCORES]
        mesh = Mesh(_np.asarray(devices), ("core",))
        n_outs = len(out_names)
        self._fn = jax.jit(
            shard_map(_body, mesh=mesh,
                      in_specs=(PartitionSpec("core"),) * (n_params + n_outs),
                      out_specs=(PartitionSpec("core"),) * n_outs,
                      check_rep=False),
            donate_argnums=tuple(range(n_params, n_params + n_outs)),
            keep_unused=True,
        )

    def prepare(self, in_maps):
        n = NCORES
        return [
            _np.concatenate([_np.asarray(in_maps[c][name]) for c in range(n)], axis=0)
            for name in self.in_names
        ]

    def run_prepared(self, concat_in):
        n = NCORES
        concat_zeros = [_np.zeros((n * z.shape[0], *z.shape[1:]), z.dtype)
                        for z in self.zero_outs]
        return self._fn(*concat_in, *concat_zeros)

    def __call__(self, in_maps):
        n = NCORES
        outs = self.run_prepared(self.prepare(in_maps))
        outs = [_np.asarray(o) for o in outs]
        return [
            {name: outs[i].reshape(n, *self.out_avals[i].shape)[c]
             for i, name in enumerate(self.out_names)}
            for c in range(n)
        ]


def _get_progs():
    if "l1" not in _progs:
        _progs["l1"] = _Executor(_build_l1())
        _progs["l2a"] = _Executor(_build_l2a())
        _progs["l2b"] = _Executor(_build_l2b())
    return _progs["l1"], _progs["l2a"], _progs["l2b"]


def kernel(xyz, w1, w2, w3, k):
    xyz = np.asarray(xyz, dtype=np.float32)
    w1 = np.asarray(w1, dtype=np.float32)
    w2 = np.asarray(w2, dtype=np.float32)
    w3 = np.asarray(w3, dtype=np.float32)
    assert int(k) == K and xyz.shape == (B, N, 3)
    l1, l2a, l2b = _get_progs()
    cores = list(range(NCORES))

    # ---- L1: coarse chunk selection -------------------------------------
    # hi/lo fp32r decomposition restores ~fp32 scoring accuracy on the PE:
    # score = qh.vh + qh.vl + ql.vh - sqh - sql  (v = 2x, sq = |x|^2)
    xyzT_b = []
    for b in range(B):
        X = xyz[b]
        sq = (X[:, 0] ** 2 + X[:, 1] ** 2 + X[:, 2] ** 2).astype(np.float32)
        v = (2.0 * X.T).astype(np.float32)                   # (3, N)
        vh, vl = _hilo(v)
        sqh, sql = _hilo(sq)
        xyzT_b.append(np.concatenate(
            [vh, vl, vh, sqh[None, :], sql[None, :]]).astype(np.float32))
    in1 = []
    for c in cores:
        b, h = c // 2, c % 2
        Q = xyz[b, h * NQ:(h + 1) * NQ]
        qh, ql = _hilo(Q.T.astype(np.float32))               # (3, NQ)
        ones = -np.ones((1, NQ), np.float32)
        qT = np.concatenate([qh, qh, ql, ones, ones]).astype(np.float32)
        in1.append({"xyzT": xyzT_b[b], "qT": qT})
    r1 = l1(in1)

    # ---- host glue: superset gather (chunk c members = c + 256*j) -------
    sup = []   # per-core (NQ, W) global candidate ids
    in2 = []
    for c in cores:
        b, h = c // 2, c % 2
        ids = r1[c]["ids"][:, :NSEL].astype(np.int64)          # (NQ, 20)
        s = (ids[:, :, None] + (np.arange(CH) * NCH)[None, None, :]).reshape(NQ, W)
        sup.append(s)
        g = xyz[b][s]                                          # (NQ, W, 3)
        g3 = np.ascontiguousarray(g.transpose(0, 2, 1)).reshape(NQ, 3 * W)
        nq3 = -np.ascontiguousarray(xyz[b, h * NQ:(h + 1) * NQ])
        in2.append({"g": g3.astype(np.float32), "nq": nq3.astype(np.float32)})
    r2 = l2a(in2)

    # ---- host glue: final-16 gather ------------------------------------
    w1blkT = np.zeros((6, 128), np.float32)
    w1blkT[0:3, 0:64] = w1.T
    w1blkT[3:6, 64:128] = w1.T
    w2blkT = np.zeros((128, 128), np.float32)
    w2blkT[0:64, 0:64] = w2.T
    w2blkT[64:128, 64:128] = w2.T
    w3blkT = np.zeros((128, 128), np.float32)
    w3blkT[0:64, 0:64] = w3.T
    w3blkT[64:128, 64:128] = w3.T
    eye = np.eye(128, dtype=np.float32)
    in3 = []
    for c in cores:
        b, h = c // 2, c % 2
        loc = r2[c]["loc"].astype(np.int64)            # (NQ, 24)
        glob = np.take_along_axis(sup[c], loc[:, 1:KK], axis=1)  # (NQ, 16)
        g16 = xyz[b][glob]                                     # (NQ, 16, 3)
        gA, gB = g16[:, 0::2, :], g16[:, 1::2, :]
        g6 = np.concatenate([gA, gB], axis=2)                  # (NQ, 8, 6)
        g6 = np.ascontiguousarray(g6.transpose(2, 0, 1)).reshape(6, NQ * 8)
        q = xyz[b, h * NQ:(h + 1) * NQ]
        xq6 = np.repeat(np.concatenate([q, q], axis=1)[:, None, :], 8, axis=1)
        xq6 = np.ascontiguousarray(xq6.transpose(2, 0, 1)).reshape(6, NQ * 8)
        in3.append({"gx12": _r32r(np.concatenate([g6, xq6], axis=0)),
                    "w12b": _r32r(np.concatenate([w1blkT, -w1blkT], axis=0)),
                    "w2b": _r32r(w2blkT), "w3b": _r32r(w3blkT), "eye": eye})
    r3 = l2b(in3)

    out = np.zeros((B, C, N), np.float32)
    for c in cores:
        b, h = c // 2, c % 2
        out[b, :, h * NQ:(h + 1) * NQ] = r3[c]["out"].T
    return out


# revision 50
# speedup vs baseline: 1.0284x; 1.0196x over previous
"""kNN (k=16) + grouped 3->64->64->64 MLP + neighbor max-pool on 8 TRN2 cores.

Pipeline (device does all O(N^2) compute, selection, and MLP flops):
  L1 : S[q,j] = 2<xq,xj> - |xj|^2 on PE as fp32r matmuls (1 cyc/col) with a
       hi/lo fp32r operand decomposition (contraction 11) restoring ~fp32
       accuracy; Act evacuates PSUM to SBUF (Pool cannot read PSUM and only
       DVE can compute max on HW); chunk-16 max via a DVE pairwise max tree
       over contiguous slabs (chunk c = points {c + 256*j}); top-24 chunk
       ids via max8/max_index/match_replace rounds on DVE, software-
       pipelined one block behind the reduce.
  host: gather the 20*16=320 candidate coords per query (index routing).
  L2A: exact squared dists in reference fp32 arithmetic on the 320-wide
       compacted domain (Act squares, Pool subtract chain); exact top-17
       (slot 0 = self) via DVE rounds -> local indices; per-block constants
       preloaded in one DMA, ids/loc written out in one batched DMA.
  L2B: relative coords via matmul-folded subtract, packed 2-point 3-layer
       MLP on PE (fp32r, host-prerounded weights/inputs), neighbor max-pool
       as single-PSUM-input DVE TensorReduce, PE transpose lagged one block,
       final pair max.

Sharding: core c handles batch c//2, query half c%2 (2048 queries each).
"""
import sys
import numpy as np

sys.path.insert(0, "/opt/trn_rl_repo")

import jax
import numpy as _np
from jax.sharding import Mesh, PartitionSpec
from jax.experimental.shard_map import shard_map

import concourse.bacc as bacc
import concourse.mybir as mybir
import concourse.tile as tile
from concourse import bass2jax
from concourse.bass2jax import _bass_exec_p, install_neuronx_cc_hook

F32 = mybir.dt.float32
F32R = mybir.dt.float32r
U16 = mybir.dt.uint16
AX = mybir.AxisListType
OP = mybir.AluOpType
AF = mybir.ActivationFunctionType

def _r32r(a):
    """Round fp32 to fp32r (11 explicit mantissa bits, round-half-up @bit 12)."""
    b = np.ascontiguousarray(a, dtype=np.float32).view(np.uint32)
    b = ((b.astype(np.uint64) + 0x800) & 0xFFFFF000).astype(np.uint32)
    return b.view(np.float32)


def _hilo(a):
    h = _r32r(a)
    return h, _r32r((a.astype(np.float32) - h).astype(np.float32))


B, N, C, K = 4, 4096, 64, 16
KK = K + 1              # 17
CH = 16                 # points per chunk
NCH = N // CH           # 256 chunks; member j of chunk c = point c + 256*j
NSEL = 20               # chunks kept per query (17 guarantee + 3 tie slack)
W = NSEL * CH           # 320 candidate superset per query
NQ = 2048               # queries per core
NBLK = NQ // 128        # 16
NEG = -1.0e30
NCORES = 8
B1P = 576               # Pool share of the B-half first max-tree level

_progs = {}


def _rounds(nc, sp, vals, out_ids, tag):
    """3x (max8 -> max_index -> match_replace) producing 24 ids, mutating vals."""
    for r in range(3):
        m8 = sp.tile([128, 8], F32, tag=f"m8{tag}", name=f"m8{tag}_{r}_{id(vals)}")
        nc.vector.max(out=m8[:], in_=vals)
        nc.vector.max_index(out=out_ids[:, r * 8:(r + 1) * 8], in_max=m8[:],
                            in_values=vals)
        if r < 2:
            nc.vector.match_replace(out=vals, in_to_replace=m8[:], in_values=vals,
                                    imm_value=NEG)


def _build_l1(repeat=1):
    nc = bacc.Bacc("TRN2", target_bir_lowering=False, debug=False,
                   num_devices=NCORES)
    xyzT_d = nc.dram_tensor("xyzT", [11, N], F32R, kind="ExternalInput").ap()
    qT_d = nc.dram_tensor("qT", [11, NQ], F32R, kind="ExternalInput").ap()
    ids_d = nc.dram_tensor("ids", [NQ, 24], U16, kind="ExternalOutput").ap()
    with tile.TileContext(nc) as tc:
        with (
            tc.tile_pool(name="tabs", bufs=1) as tabs,
            tc.tile_pool(name="psum", bufs=1, space="PSUM") as pp,
            tc.tile_pool(name="workbig", bufs=3) as wb,
            tc.tile_pool(name="work", bufs=6) as wp,
            tc.tile_pool(name="small", bufs=8) as sp,
        ):
            xyzT_sb = tabs.tile([11, N], F32R)
            qT_sb = tabs.tile([11, NQ], F32R)
            warm = tabs.tile([11, 512], F32)
            ids_all = tabs.tile([128, NBLK * 24], U16)
            nc.vector.memset(warm[:], 0.0)
            nc.sync.dma_start(out=xyzT_sb[:], in_=xyzT_d[:])
            nc.sync.dma_start(out=qT_sb[:], in_=qT_d[:])
            # ramp the PE to full clock while the inputs stream in
            pswarm = pp.tile([128, 2048], F32, tag="psA", name="pswarm")
            for n in range(7):
                nc.tensor.matmul(pswarm[:, 0:512], warm[:, 0:128], warm[:],
                                 start=True, stop=True)
            c16s = {}
            nblk = repeat * NBLK
            for i in range(nblk):
                ib = i % NBLK
                lhsT = qT_sb[:, ib * 128:(ib + 1) * 128]
                psA = pp.tile([128, 2048], F32, tag="psA", name=f"psA_{i}")
                psB = pp.tile([128, 2048], F32, tag="psB", name=f"psB_{i}")
                for k in range(4):
                    nc.tensor.matmul(psA[:, k * 512:(k + 1) * 512], lhsT,
                                     xyzT_sb[:, k * 512:(k + 1) * 512],
                                     start=True, stop=True)
                for k in range(4):
                    nc.tensor.matmul(psB[:, k * 512:(k + 1) * 512], lhsT,
                                     xyzT_sb[:, (4 + k) * 512:(5 + k) * 512],
                                     start=True, stop=True)
                # HW: Pool can't read PSUM and ops take at most one PSUM
                # operand, so Act (otherwise idle) evacuates PSUM to SBUF and
                # the pair-max tree runs in SBUF. Pairings keep chunk=pt%256.
                cpB = wb.tile([128, 2048], F32, tag="cpB", name=f"cpB_{i}")
                nc.scalar.activation(cpB[:], psB[:], AF.Copy)
                s1 = wb.tile([128, 2048], F32, tag="s1", name=f"s1_{i}")
                nc.vector.tensor_tensor(s1[:], psA[:], cpB[:], op=OP.max)
                s2 = wp.tile([128, 1024], F32, tag="s2", name=f"s2_{i}")
                nc.vector.tensor_tensor(s2[:], s1[:, 0:1024], s1[:, 1024:2048],
                                        op=OP.max)
                s3 = wp.tile([128, 512], F32, tag="s3", name=f"s3_{i}")
                nc.vector.tensor_tensor(s3[:], s2[:, 0:512], s2[:, 512:1024],
                                        op=OP.max)
                c16 = wp.tile([128, NCH], F32, tag="c16", name=f"c16_{i}")
                nc.vector.tensor_tensor(c16[:], s3[:, 0:256], s3[:, 256:512],
                                        op=OP.max)
                c16s[i] = c16
                # software pipelining: selection rounds lag the reduce by one
                # block so the DVE->Pool->DVE cycle spans two periods
                if i >= 1:
                    j = i - 1
                    _rounds(nc, sp, c16s.pop(j)[:],
                            ids_all[:, (j % NBLK) * 24:(j % NBLK + 1) * 24], "a")
                    if j == NBLK // 2 - 1:
                        nc.sync.dma_start(
                            out=ids_d[0:NQ // 2].rearrange("(i p) s -> p i s", p=128),
                            in_=ids_all[:, 0:NBLK // 2 * 24].rearrange(
                                "p (i s) -> p i s", s=24))
            j = nblk - 1
            _rounds(nc, sp, c16s.pop(j)[:],
                    ids_all[:, (j % NBLK) * 24:(j % NBLK + 1) * 24], "a")
            nc.sync.dma_start(
                out=ids_d[NQ // 2:NQ].rearrange("(i p) s -> p i s", p=128),
                in_=ids_all[:, NBLK // 2 * 24:].rearrange("p (i s) -> p i s", s=24))
    nc.compile()
    return nc


def _build_l2a(repeat=1):
    nc = bacc.Bacc("TRN2", target_bir_lowering=False, debug=False,
                   num_devices=NCORES)
    g_d = nc.dram_tensor("g", [NQ, 3 * W], F32, kind="ExternalInput").ap()
    nq_d = nc.dram_tensor("nq", [NQ, 3], F32, kind="ExternalInput").ap()
    loc_d = nc.dram_tensor("loc", [NQ, 24], U16, kind="ExternalOutput").ap()
    with tile.TileContext(nc) as tc:
        with (
            tc.tile_pool(name="tabs", bufs=1) as tabs,
            tc.tile_pool(name="work", bufs=6) as wp,
            tc.tile_pool(name="small", bufs=6) as sp,
        ):
            nq_all = tabs.tile([128, NBLK * 3], F32)
            loc_all = tabs.tile([128, NBLK * 24], U16)
            zz = tabs.tile([128, W], F32)
            nc.vector.memset(zz[:], 0.0)
            gt0 = wp.tile([128, 3 * W], F32, tag="gt", name="gt_0")
            nc.sync.dma_start(out=gt0[:], in_=g_d[0:128, :])
            nc.sync.dma_start(
                out=nq_all[:].rearrange("p (i c) -> p i c", c=3),
                in_=nq_d[:].rearrange("(i p) c -> p i c", p=128))
            for i in range(repeat * NBLK):
                ib = i % NBLK
                sl = slice(ib * 128, (ib + 1) * 128)
                if i == 0:
                    gt = gt0
                else:
                    gt = wp.tile([128, 3 * W], F32, tag="gt", name=f"gt_{i}")
                    nc.sync.dma_start(out=gt[:], in_=g_d[sl, :])
                nsq = wp.tile([128, 3, W], F32, tag="nsq", name=f"nsq_{i}")
                for c in range(3):
                    nc.scalar.activation(nsq[:, c, :], gt[:, c * W:(c + 1) * W],
                                         AF.Square,
                                         bias=nq_all[:, ib * 3 + c:ib * 3 + c + 1],
                                         scale=1.0)
                # nd = ((0-sq0)-sq1)-sq2, bit-identical to reference order
                nd = wp.tile([128, W], F32, tag="nd", name=f"nd_{i}")
                nc.gpsimd.tensor_tensor(nd[:], zz[:], nsq[:, 0, :], op=OP.subtract)
                nc.gpsimd.tensor_tensor(nd[:], nd[:], nsq[:, 1, :], op=OP.subtract)
                nc.gpsimd.tensor_tensor(nd[:], nd[:], nsq[:, 2, :], op=OP.subtract)
                _rounds(nc, sp, nd[:], loc_all[:, ib * 24:(ib + 1) * 24], "b")
                if ib == NBLK // 2 - 1:
                    nc.sync.dma_start(
                        out=loc_d[0:NQ // 2].rearrange("(i p) s -> p i s", p=128),
                        in_=loc_all[:, 0:NBLK // 2 * 24].rearrange(
                            "p (i s) -> p i s", s=24))
            nc.sync.dma_start(
                out=loc_d[NQ // 2:NQ].rearrange("(i p) s -> p i s", p=128),
                in_=loc_all[:, NBLK // 2 * 24:].rearrange("p (i s) -> p i s", s=24))
    nc.compile()
    return nc


def _build_l2b(repeat=1):
    nc = bacc.Bacc("TRN2", target_bir_lowering=False, debug=False,
                   num_devices=NCORES)
    gx_d = nc.dram_tensor("gx12", [12, NQ * 8], F32R, kind="ExternalInput").ap()
    w12_d = nc.dram_tensor("w12b", [12, 128], F32R, kind="ExternalInput").ap()
    w2_d = nc.dram_tensor("w2b", [128, 128], F32R, kind="ExternalInput").ap()
    w3_d = nc.dram_tensor("w3b", [128, 128], F32R, kind="ExternalInput").ap()
    eye_d = nc.dram_tensor("eye", [128, 128], F32, kind="ExternalInput").ap()
    out_d = nc.dram_tensor("out", [NQ, C], F32, kind="ExternalOutput").ap()
    with tile.TileContext(nc) as tc:
        with (
            tc.tile_pool(name="tabs", bufs=1) as tabs,
            tc.tile_pool(name="psum", bufs=1, space="PSUM") as pp,
            tc.tile_pool(name="psumT", bufs=2, space="PSUM") as ppt,
            tc.tile_pool(name="work", bufs=4) as wp,
            tc.tile_pool(name="small", bufs=4) as sp,
        ):
            w12_sb = tabs.tile([12, 128], F32R)
            w2_sb = tabs.tile([128, 128], F32R)
            w3_sb = tabs.tile([128, 128], F32R)
            eye_sb = tabs.tile([128, 128], F32)
            gx_sb = tabs.tile([12, NQ * 8], F32R)
            fin_all = tabs.tile([128, NBLK * 64], F32)
            for sb, dd in ((w12_sb, w12_d), (gx_sb, gx_d), (w2_sb, w2_d),
                           (w3_sb, w3_d), (eye_sb, eye_d)):
                nc.sync.dma_start(out=sb[:], in_=dd[:])
            warm = tabs.tile([12, 512], F32)
            nc.vector.memset(warm[:], 0.0)
            pswarm = pp.tile([128, 512], F32, tag="ps10", name="pswarm")
            for n in range(8):
                nc.tensor.matmul(pswarm[:], warm[:, 0:128], warm[:],
                                 start=True, stop=True)
            mxs = {}

            def _tail(j):
                jb = j % NBLK
                pst = ppt.tile([128, 128], F32, tag="pst", name=f"pst_{j}")
                nc.tensor.transpose(pst[:], mxs.pop(j)[:], eye_sb[:])
                mxT = sp.tile([128, 128], F32, tag="mxT", name=f"mxT_{j}")
                nc.scalar.activation(mxT[:], pst[:], AF.Copy)
                nc.vector.tensor_tensor(fin_all[:, jb * 64:(jb + 1) * 64],
                                        mxT[:, 0:64], mxT[:, 64:128], op=OP.max)

            for i in range(repeat * NBLK):
                ib = i % NBLK
                mx = sp.tile([128, 128], F32, tag="mx", name=f"mx_{i}")
                cs = [slice(ib * 1024 + t * 512, ib * 1024 + (t + 1) * 512)
                      for t in range(2)]
                # interleave the two independent halves so their chains run
                # concurrently on different engines
                ps1 = [pp.tile([128, 512], F32, tag=f"ps1{t}", name=f"ps1_{i}_{t}")
                       for t in range(2)]
                for t in range(2):
                    nc.tensor.matmul(ps1[t][:], w12_sb[:], gx_sb[:, cs[t]],
                                     start=True, stop=True)
                h1 = [wp.tile([128, 512], F32R, tag=f"h1{t}", name=f"h1_{i}_{t}")
                      for t in range(2)]
                nc.scalar.activation(h1[0][:], ps1[0][:], AF.Relu)
                nc.vector.tensor_scalar(h1[1][:], ps1[1][:], 0.0, scalar2=None,
                                        op0=OP.max)
                ps2 = [pp.tile([128, 512], F32, tag=f"ps2{t}", name=f"ps2_{i}_{t}")
                       for t in range(2)]
                for t in range(2):
                    nc.tensor.matmul(ps2[t][:], w2_sb[:], h1[t][:],
                                     start=True, stop=True)
                h2 = [wp.tile([128, 512], F32R, tag=f"h2{t}", name=f"h2_{i}_{t}")
                      for t in range(2)]
                nc.scalar.activation(h2[0][:], ps2[0][:], AF.Relu)
                nc.scalar.activation(h2[1][:], ps2[1][:], AF.Relu)
                ps3 = [pp.tile([128, 512], F32, tag=f"ps3{t}", name=f"ps3_{i}_{t}")
                       for t in range(2)]
                for t in range(2):
                    nc.tensor.matmul(ps3[t][:], w3_sb[:], h2[t][:],
                                     start=True, stop=True)
                # neighbor max-pool: single-PSUM-input TensorReduce on DVE
                for t in range(2):
                    nc.vector.tensor_reduce(
                        mx[:, t * 64:(t + 1) * 64],
                        ps3[t][:].rearrange("m (q p) -> m q p", p=8),
                        axis=AX.X, op=OP.max)
                mxs[i] = mx
                # transpose+final-max lag one block so the PE stream is never
                # head-of-line blocked on the current block's pool tree
                if i >= 1:
                    _tail(i - 1)
                    if i - 1 == NBLK // 2 - 1:
                        nc.sync.dma_start(
                            out=out_d[0:NQ // 2].rearrange("(i p) c -> p i c", p=128),
                            in_=fin_all[:, 0:NBLK // 2 * 64].rearrange(
                                "p (i c) -> p i c", c=64))
            _tail(repeat * NBLK - 1)
            nc.sync.dma_start(
                out=out_d[NQ // 2:NQ].rearrange("(i p) c -> p i c", p=128),
                in_=fin_all[:, NBLK // 2 * 64:].rearrange("p (i c) -> p i c", c=64))
    nc.compile()
    return nc


class _Executor:
    """Cached multi-core PJRT executor for one prebuilt Bass program."""

    def __init__(self, nc):
        install_neuronx_cc_hook()
        self.nc = nc
        part_name = nc.partition_id_tensor.name if nc.partition_id_tensor else None
        in_names, out_names, out_avals, zero_outs = [], [], [], []
        for alloc in nc.m.functions[0].allocations:
            if not isinstance(alloc, mybir.MemoryLocationSet):
                continue
            name = alloc.memorylocations[0].name
            if alloc.kind == "ExternalInput":
                if name != part_name:
                    in_names.append(name)
            elif alloc.kind == "ExternalOutput":
                shape = tuple(alloc.tensor_shape)
                dtype = mybir.dt.np(alloc.dtype)
                out_names.append(name)
                out_avals.append(jax.core.ShapedArray(shape, dtype))
                zero_outs.append(_np.zeros(shape, dtype))
        self.in_names, self.out_names = in_names, out_names
        self.out_avals, self.zero_outs = out_avals, zero_outs
        n_params = len(in_names)
        all_names = in_names + out_names
        if part_name is not None:
            all_names = all_names + [part_name]

        def _body(*args):
            operands = list(args)
            if part_name is not None:
                operands.append(bass2jax.partition_id_tensor())
            return tuple(_bass_exec_p.bind(
                *operands,
                out_avals=tuple(out_avals),
                in_names=tuple(all_names),
                out_names=tuple(out_names),
                lowering_input_output_aliases=(),
                sim_require_finite=True,
                sim_require_nnan=True,
                nc=nc,
            ))

        devices = jax.devices()[:NCORES]
        mesh = Mesh(_np.asarray(devices), ("core",))
        n_outs = len(out_names)
        self._fn = jax.jit(
            shard_map(_body, mesh=mesh,
                      in_specs=(PartitionSpec("core"),) * (n_params + n_outs),
                      out_specs=(PartitionSpec("core"),) * n_outs,
                      check_rep=False),
            donate_argnums=tuple(range(n_params, n_params + n_outs)),
            keep_unused=True,
        )

    def prepare(self, in_maps):
        n = NCORES
        return [
            _np.concatenate([_np.asarray(in_maps[c][name]) for c in range(n)], axis=0)
            for name in self.in_names
        ]

    def run_prepared(self, concat_in):
        n = NCORES
        concat_zeros = [_np.zeros((n * z.shape[0], *z.shape[1:]), z.dtype)
                        for z in self.zero_outs]
        return self._fn(*concat_in, *concat_zeros)

    def __call__(self, in_maps):
        n = NCORES
        outs = self.run_prepared(self.prepare(in_maps))
        outs = [_np.asarray(o) for o in outs]
        return [
            {name: outs[i].reshape(n, *self.out_avals[i].shape)[c]
             for i, name in enumerate(self.out_names)}
            for c in range(n)
        ]


def _get_progs():
    if "l1" not in _progs:
        _progs["l1"] = _Executor(_build_l1())
        _progs["l2a"] = _Executor(_build_l2a())
        _progs["l2b"] = _Executor(_build_l2b())
    return _progs["l1"], _progs["l2a"], _progs["l2b"]


def kernel(xyz, w1, w2, w3, k):
    xyz = np.asarray(xyz, dtype=np.float32)
    w1 = np.asarray(w1, dtype=np.float32)
    w2 = np.asarray(w2, dtype=np.float32)
    w3 = np.asarray(w3, dtype=np.float32)
    assert int(k) == K and xyz.shape == (B, N, 3)
    l1, l2a, l2b = _get_progs()
    cores = list(range(NCORES))

    # ---- L1: coarse chunk selection -------------------------------------
    # hi/lo fp32r decomposition restores ~fp32 scoring accuracy on the PE:
    # score = qh.vh + qh.vl + ql.vh - sqh - sql  (v = 2x, sq = |x|^2)
    xyzT_b = []
    for b in range(B):
        X = xyz[b]
        sq = (X[:, 0] ** 2 + X[:, 1] ** 2 + X[:, 2] ** 2).astype(np.float32)
        v = (2.0 * X.T).astype(np.float32)                   # (3, N)
        vh, vl = _hilo(v)
        sqh, sql = _hilo(sq)
        xyzT_b.append(np.concatenate(
            [vh, vl, vh, sqh[None, :], sql[None, :]]).astype(np.float32))
    in1 = []
    for c in cores:
        b, h = c // 2, c % 2
        Q = xyz[b, h * NQ:(h + 1) * NQ]
        qh, ql = _hilo(Q.T.astype(np.float32))               # (3, NQ)
        ones = -np.ones((1, NQ), np.float32)
        qT = np.concatenate([qh, qh, ql, ones, ones]).astype(np.float32)
        in1.append({"xyzT": xyzT_b[b], "qT": qT})
    r1 = l1(in1)

    # ---- host glue: superset gather (chunk c members = c + 256*j) -------
    sup = []   # per-core (NQ, W) global candidate ids
    in2 = []
    for c in cores:
        b, h = c // 2, c % 2
        ids = r1[c]["ids"][:, :NSEL].astype(np.int64)          # (NQ, 20)
        s = (ids[:, :, None] + (np.arange(CH) * NCH)[None, None, :]).reshape(NQ, W)
        sup.append(s)
        g = xyz[b][s]                                          # (NQ, W, 3)
        g3 = np.ascontiguousarray(g.transpose(0, 2, 1)).reshape(NQ, 3 * W)
        nq3 = -np.ascontiguousarray(xyz[b, h * NQ:(h + 1) * NQ])
        in2.append({"g": g3.astype(np.float32), "nq": nq3.astype(np.float32)})
    r2 = l2a(in2)

    # ---- host glue: final-16 gather ------------------------------------
    w1blkT = np.zeros((6, 128), np.float32)
    w1blkT[0:3, 0:64] = w1.T
    w1blkT[3:6, 64:128] = w1.T
    w2blkT = np.zeros((128, 128), np.float32)
    w2blkT[0:64, 0:64] = w2.T
    w2blkT[64:128, 64:128] = w2.T
    w3blkT = np.zeros((128, 128), np.float32)
    w3blkT[0:64, 0:64] = w3.T
    w3blkT[64:128, 64:128] = w3.T
    eye = np.eye(128, dtype=np.float32)
    in3 = []
    for c in cores:
        b, h = c // 2, c % 2
        loc = r2[c]["loc"].astype(np.int64)            # (NQ, 24)
        glob = np.take_along_axis(sup[c], loc[:, 1:KK], axis=1)  # (NQ, 16)
        g16 = xyz[b][glob]                                     # (NQ, 16, 3)
        gA, gB = g16[:, 0::2, :], g16[:, 1::2, :]
        g6 = np.concatenate([gA, gB], axis=2)                  # (NQ, 8, 6)
        g6 = np.ascontiguousarray(g6.transpose(2, 0, 1)).reshape(6, NQ * 8)
        q = xyz[b, h * NQ:(h + 1) * NQ]
        xq6 = np.repeat(np.concatenate([q, q], axis=1)[:, None, :], 8, axis=1)
        xq6 = np.ascontiguousarray(xq6.transpose(2, 0, 1)).reshape(6, NQ * 8)
        in3.append({"gx12": _r32r(np.concatenate([g6, xq6], axis=0)),
                    "w12b": _r32r(np.concatenate([w1blkT, -w1blkT], axis=0)),
                    "w2b": _r32r(w2blkT), "w3b": _r32r(w3blkT), "eye": eye})
    r3 = l2b(in3)

    out = np.zeros((B, C, N), np.float32)
    for c in cores:
        b, h = c // 2, c % 2
        out[b, :, h * NQ:(h + 1) * NQ] = r3[c]["out"].T
    return out
